# revision 3
# baseline (speedup 1.0000x reference)
"""Trainium2 Bass kernel for nn_AttentionStem (sparse local attention stem).

Math per output element (b, c, h, w), window kk = (di, dj) in 4x4, PAD=2:
  E[c,h,w]   = (emb_a[c,w] + emb_b[c,h]) * emb_mix[c,h,w]
  e1_kk      = exp(v_kk^2 * E)                  (softmax-1 numerator)
  q'         = q / sum_kk(e1)                   (fold softmax-1 denom into q)
  e2_kk      = exp(q' * k_kk * e1_kk)           (softmax-2 numerator)
  out        = sum_kk(e2 * v_kk) / sum_kk(e2)

Sharding: pure data parallel, one batch element per NeuronCore (8 cores).
Layout per core: SBUF partition p = 64*half + c  (half = h<64 ? 0 : 1),
free dims stream (h, w); KK tiles are [128, KK, n] with kk outermost.
"""
import sys, os
for _p in ("/opt/trn_rl_repo", "/root/.axon_site/_ro/trn_rl_repo"):
    if os.path.isdir(_p) and _p not in sys.path:
        sys.path.insert(0, _p)

from contextlib import ExitStack
import numpy as np

import concourse.bass as bass
import concourse.bacc as bacc
import concourse.tile as tile
from concourse import mybir
import concourse.bass_utils as bass_utils
from concourse.bass_types import AP

N_CORES = 8
B, CIN, H, W = 8, 3, 128, 128
C = 64
K, PAD, KK = 4, 2, 16
HP, WP = H + 2 * PAD, W + 2 * PAD  # 132, 132
HH = H // 2                        # rows per half (64)

F32 = mybir.dt.float32
MULT = mybir.AluOpType.mult
ADD = mybir.AluOpType.add

CH = 2  # h-rows per half per chunk


def _ap(base: AP, offset: int, dims):
    """Build a custom free-dim AP on a tile/dram AP, keeping its partition dim."""
    return AP(tensor=base.tensor, offset=base.offset + offset,
              ap=[list(base.ap[0])] + [list(d) for d in dims])


def _win(base: AP, row_stride: int, ch: int, n_w: int = W):
    """Shifted-window read AP: dims [di, dj, h, w] over a padded row-major map."""
    return _ap(base, 0, [[row_stride, K], [1, K], [row_stride, ch], [1, n_w]])


def build_kernel(nc, ch: int = CH):
    f32 = F32
    n = ch * W                      # spatial elems per partition per chunk
    RR = ch + K - 1                 # padded rows needed per half per chunk
    RW = RR * WP                    # map cols per half per chunk
    piece = (RW + 1) // 2 if RW > 512 else RW   # matmul col piece (<=512)

    xp_d = nc.dram_tensor("xp", [CIN, HP * WP], f32, kind="ExternalInput").ap()
    qw_d = nc.dram_tensor("q_wT", [CIN, C], f32, kind="ExternalInput").ap()
    kw_d = nc.dram_tensor("k_wT", [CIN, C], f32, kind="ExternalInput").ap()
    vw_d = nc.dram_tensor("v_wT", [CIN, C], f32, kind="ExternalInput").ap()
    ea_d = nc.dram_tensor("emb_a", [C, W], f32, kind="ExternalInput").ap()
    eb_d = nc.dram_tensor("emb_b", [C, H], f32, kind="ExternalInput").ap()
    em_d = nc.dram_tensor("emb_mix", [C, H * W], f32, kind="ExternalInput").ap()
    out_d = nc.dram_tensor("out", [C, H * W], f32, kind="ExternalOutput").ap()

    with tile.TileContext(nc) as tc, ExitStack() as ctx:
        const = ctx.enter_context(tc.tile_pool(name="const", bufs=1))
        xp_p = ctx.enter_context(tc.tile_pool(name="xp", bufs=3))
        mix_p = ctx.enter_context(tc.tile_pool(name="mix", bufs=3))
        map_p = ctx.enter_context(tc.tile_pool(name="maps", bufs=2))
        kk_p = ctx.enter_context(tc.tile_pool(name="kk", bufs=2))
        sm_p = ctx.enter_context(tc.tile_pool(name="small", bufs=2))
        ps_kv = ctx.enter_context(tc.tile_pool(name="pskv", bufs=6, space="PSUM"))
        ps_q = ctx.enter_context(tc.tile_pool(name="psq", bufs=2, space="PSUM"))

        # ---- constants ----
        qw_t = const.tile([CIN, C], f32, tag="qw")
        kw_t = const.tile([CIN, C], f32, tag="kw")
        vw_t = const.tile([CIN, C], f32, tag="vw")
        nc.sync.dma_start(qw_t[:], qw_d[:])
        nc.sync.dma_start(kw_t[:], kw_d[:])
        nc.sync.dma_start(vw_t[:], vw_d[:])
        ea_t = const.tile([128, W], f32, tag="ea")       # emb_a[c, w], both halves
        nc.sync.dma_start(ea_t[0:C, :], ea_d[:])
        nc.sync.dma_start(ea_t[C:128, :], ea_d[:])
        eb_t = const.tile([128, HH], f32, tag="eb")      # emb_b[c, 64*half + hl]
        nc.sync.dma_start(eb_t[0:C, :], _ap(eb_d, 0, [[1, HH]]))
        nc.sync.dma_start(eb_t[C:128, :], _ap(eb_d, HH, [[1, HH]]))

        for h0 in range(0, HH, ch):
            # ---- input chunk DMAs ----
            xp_t = xp_p.tile([CIN, 2 * RW], f32, tag="xp")
            for half in (0, 1):
                nc.sync.dma_start(
                    xp_t[:, half * RW:(half + 1) * RW],
                    _ap(xp_d, (HH * half + h0) * WP, [[1, RW]]))
            mix_t = mix_p.tile([128, n], f32, tag="mix")
            for half in (0, 1):
                nc.sync.dma_start(
                    mix_t[C * half:C * (half + 1), :],
                    _ap(em_d, (HH * half + h0) * W, [[1, n]]))

            # ---- 1x1 convs on PE ----
            q_ps = ps_q.tile([128, n], f32, tag="q")
            for half in (0, 1):
                rhs = _ap(xp_t[:], half * RW + PAD * WP + PAD, [[WP, ch], [1, W]])
                nc.tensor.matmul(q_ps[C * half:C * (half + 1), :], qw_t[:], rhs,
                                 start=True, stop=True)
            kv_ps = {}
            for name, w_t in (("k", kw_t), ("v", vw_t)):
                for pc in range(0, RW, piece):
                    pw = min(piece, RW - pc)
                    pt = ps_kv.tile([128, piece], f32, tag="kv")
                    kv_ps[(name, pc)] = (pt, pw)
                    for half in (0, 1):
                        nc.tensor.matmul(
                            pt[C * half:C * (half + 1), 0:pw],
                            w_t[:], xp_t[:, half * RW + pc: half * RW + pc + pw],
                            start=True, stop=True)

            # ---- PSUM -> SBUF map copies (ACT), v2 on DVE ----
            k_map = map_p.tile([128, RW], f32, tag="kmap")
            v_map = map_p.tile([128, RW], f32, tag="vmap")
            for name, m_t in (("k", k_map), ("v", v_map)):
                for pc in range(0, RW, piece):
                    pt, pw = kv_ps[(name, pc)]
                    nc.scalar.copy(m_t[:, pc:pc + pw], pt[:, 0:pw])
            v2_map = map_p.tile([128, RW], f32, tag="v2map")
            nc.vector.tensor_tensor(v2_map[:], v_map[:], v_map[:], MULT)

            # ---- E = (emb_a + emb_b) * emb_mix ----
            tmp_t = sm_p.tile([128, n], f32, tag="tmpE")
            nc.vector.tensor_tensor(
                _ap(tmp_t[:], 0, [[W, ch], [1, W]]),
                _ap(ea_t[:], 0, [[0, ch], [1, W]]),
                _ap(eb_t[:], h0, [[1, ch], [0, W]]), ADD)
            E_t = sm_p.tile([128, n], f32, tag="E")
            nc.vector.tensor_tensor(E_t[:], tmp_t[:], mix_t[:], MULT)

            # ---- KK-expanded stages ----
            # ISA allows max 3 free dims per AP -> one instruction per di.
            kk_di = [[n, K], [W, ch], [1, W]]                 # out [dj, h, w] slice
            bc_di = [[0, K], [W, ch], [1, W]]                 # center broadcast
            red_in = [[1, n], [n, KK]]                        # [pos, kk] innermost=kk

            def tt_kk(out_t, make_in0, make_in1):
                for di in range(K):
                    nc.vector.tensor_tensor(
                        _ap(out_t[:], di * K * n, kk_di),
                        make_in0(di), make_in1(di), MULT)

            def win_di(m_t):
                return lambda di: _ap(m_t[:], di * WP, [[1, K], [WP, ch], [1, W]])

            def bc_c(c_t):
                return lambda di: _ap(c_t[:], 0, bc_di)

            def kk_slice(k_t):
                return lambda di: _ap(k_t[:], di * K * n, kk_di)

            t1 = kk_p.tile([128, KK * n], f32, tag="kkA")
            tt_kk(t1, win_di(v2_map), bc_c(E_t))
            e1 = kk_p.tile([128, KK * n], f32, tag="kkB")
            nc.scalar.activation(e1[:], t1[:], mybir.ActivationFunctionType.Exp)

            r1 = sm_p.tile([128, n], f32, tag="r1")
            nc.vector.tensor_reduce(r1[:], _ap(e1[:], 0, red_in),
                                    axis=mybir.AxisListType.X, op=ADD)
            rc1 = sm_p.tile([128, n], f32, tag="rc1")
            nc.vector.reciprocal(rc1[:], r1[:])
            qp_t = sm_p.tile([128, n], f32, tag="qp")
            nc.vector.tensor_tensor(qp_t[:], q_ps[:], rc1[:], MULT)

            m1 = kk_p.tile([128, KK * n], f32, tag="kkA")
            tt_kk(m1, win_di(k_map), kk_slice(e1))
            s2 = kk_p.tile([128, KK * n], f32, tag="kkB")
            tt_kk(s2, kk_slice(m1), bc_c(qp_t))
            e2 = kk_p.tile([128, KK * n], f32, tag="kkC")
            nc.scalar.activation(e2[:], s2[:], mybir.ActivationFunctionType.Exp)

            r2 = sm_p.tile([128, n], f32, tag="r2")
            nc.vector.tensor_reduce(r2[:], _ap(e2[:], 0, red_in),
                                    axis=mybir.AxisListType.X, op=ADD)
            m2 = kk_p.tile([128, KK * n], f32, tag="kkA")
            tt_kk(m2, kk_slice(e2), win_di(v_map))
            r3 = sm_p.tile([128, n], f32, tag="r3")
            nc.vector.tensor_reduce(r3[:], _ap(m2[:], 0, red_in),
                                    axis=mybir.AxisListType.X, op=ADD)
            rc2 = sm_p.tile([128, n], f32, tag="rc2")
            nc.vector.reciprocal(rc2[:], r2[:])
            out_t = sm_p.tile([128, n], f32, tag="out")
            nc.vector.tensor_tensor(out_t[:], r3[:], rc2[:], MULT)

            for half in (0, 1):
                nc.sync.dma_start(
                    _ap(out_d, (HH * half + h0) * W, [[1, n]]),
                    out_t[C * half:C * (half + 1), :])


_compiled_nc = None


def _get_nc():
    global _compiled_nc
    if _compiled_nc is None:
        nc = bacc.Bacc("TRN2", target_bir_lowering=False, debug=False,
                       num_devices=N_CORES)
        build_kernel(nc)
        nc.compile()
        _compiled_nc = nc
    return _compiled_nc


def _shard_inputs(x, q_w, k_w, v_w, emb_a, emb_b, emb_mix):
    xp = np.pad(x.astype(np.float32), ((0, 0), (0, 0), (PAD, PAD), (PAD, PAD)))
    common = {
        "q_wT": np.ascontiguousarray(q_w.T.astype(np.float32)),
        "k_wT": np.ascontiguousarray(k_w.T.astype(np.float32)),
        "v_wT": np.ascontiguousarray(v_w.T.astype(np.float32)),
        "emb_a": np.ascontiguousarray(emb_a.astype(np.float32)),
        "emb_b": np.ascontiguousarray(emb_b.astype(np.float32)),
        "emb_mix": np.ascontiguousarray(emb_mix.astype(np.float32).reshape(C, H * W)),
    }
    return [dict(common, xp=np.ascontiguousarray(xp[b].reshape(CIN, HP * WP)))
            for b in range(B)]


def kernel(x, q_w, k_w, v_w, emb_a, emb_b, emb_mix):
    nc = _get_nc()
    in_maps = _shard_inputs(x, q_w, k_w, v_w, emb_a, emb_b, emb_mix)
    res = bass_utils.run_bass_kernel_spmd(nc, in_maps, list(range(N_CORES)))
    out = np.stack([res.results[b]["out"].reshape(C, H, W) for b in range(B)])
    return out.astype(np.float32)


# revision 15
# speedup vs baseline: 1.9274x; 1.9274x over previous
"""Trainium2 Bass kernel for nn_AttentionStem (sparse local attention stem).

Math per output element (b, c, h, w), window kk = (di, dj) in 4x4, PAD=2:
  E[c,h,w]   = (emb_a[c,w] + emb_b[c,h]) * emb_mix[c,h,w]
  e1_kk      = exp(v_kk^2 * E)                  (softmax-1 numerator)
  q'         = q / sum_kk(e1)                   (fold softmax-1 denom into q)
  e2_kk      = exp(q' * k_kk * e1_kk)           (softmax-2 numerator)
  out        = sum_kk(e2 * v_kk) / sum_kk(e2)

Sharding: pure data parallel, one batch element per NeuronCore (8 cores).
Layout per core: SBUF partition p = 64*half + c  (half = h<64 ? 0 : 1),
free dims stream (h, w); KK tiles are [128, KK, n] with kk outermost.
The three sum_kk reductions run on the TensorEngine as chains of 16
accumulating 128x128 transposes into PSUM (exact fp32 sums); softmax-2's
divide happens in transposed space and one transpose brings the result back.
"""
import sys, os
for _p in ("/opt/trn_rl_repo", "/root/.axon_site/_ro/trn_rl_repo"):
    if os.path.isdir(_p) and _p not in sys.path:
        sys.path.insert(0, _p)

from contextlib import ExitStack
import numpy as np

import concourse.bass as bass
import concourse.bacc as bacc
import concourse.tile as tile
from concourse import mybir
import concourse.bass_utils as bass_utils
from concourse.bass_types import AP
from concourse import masks

N_CORES = 8
B, CIN, H, W = 8, 3, 128, 128
C = 64
K, PAD, KK = 4, 2, 16
HP, WP = H + 2 * PAD, W + 2 * PAD  # 132, 132
HH = H // 2                        # rows per half (64)

F32 = mybir.dt.float32
BF16 = mybir.dt.bfloat16
F32R = mybir.dt.float32r
MULT = mybir.AluOpType.mult
ADD = mybir.AluOpType.add
EXP = mybir.ActivationFunctionType.Exp
SQUARE = mybir.ActivationFunctionType.Square

CH = 2  # h-rows per half per chunk

# Precision / engine configuration.
#   kk:    dtype of the KK-expanded pipeline (maps, t1/e1/m1/s2/e2/m2)
#   conv:  dtype of the 1x1-conv matmuls (fp32: 4 cyc/row, fp32r/bf16: 1)
#   e2_fp32: keep softmax-2 numerators in fp32 (accuracy of the output path)
CFG = dict(kk=BF16, conv=F32R, e2_fp32=False)


def _ap(base: AP, offset: int, dims):
    """Build a custom free-dim AP on a tile/dram AP, keeping its partition dim."""
    return AP(tensor=base.tensor, offset=base.offset + offset,
              ap=[list(base.ap[0])] + [list(d) for d in dims])


def build_kernel(nc, ch: int = CH, cfg=None):
    cfg = dict(CFG if cfg is None else cfg)
    f32 = F32
    dkk = cfg["kk"]                 # dtype of KK pipeline
    d_e2 = f32 if cfg["e2_fp32"] else dkk
    dcv = cfg["conv"]               # conv matmul dtype
    split = dkk != f32              # parity-split DVE instrs for 2x mode
    n = ch * W                      # spatial elems per partition per chunk
    NG = n // 128                   # 128-wide position groups (= ch)
    RR = ch + K - 1                 # padded rows needed per half per chunk
    RW = RR * WP                    # map cols per half per chunk
    piece = (RW + 1) // 2 if RW > 512 else RW   # matmul col piece (<=512)

    xp_d = nc.dram_tensor("xp", [CIN, HP * WP], dcv, kind="ExternalInput").ap()
    qw_d = nc.dram_tensor("q_wT", [CIN, C], dcv, kind="ExternalInput").ap()
    kw_d = nc.dram_tensor("k_wT", [CIN, C], dcv, kind="ExternalInput").ap()
    vw_d = nc.dram_tensor("v_wT", [CIN, C], dcv, kind="ExternalInput").ap()
    ea_d = nc.dram_tensor("emb_a", [C, W], f32, kind="ExternalInput").ap()
    eb_d = nc.dram_tensor("emb_b", [C, H], f32, kind="ExternalInput").ap()
    em_d = nc.dram_tensor("emb_mix", [C, H * W], f32, kind="ExternalInput").ap()
    out_d = nc.dram_tensor("out", [C, H * W], f32, kind="ExternalOutput").ap()

    with tile.TileContext(nc) as tc, ExitStack() as ctx:
        const = ctx.enter_context(tc.tile_pool(name="const", bufs=1))
        xp_p = ctx.enter_context(tc.tile_pool(name="xp", bufs=3))
        mix_p = ctx.enter_context(tc.tile_pool(name="mix", bufs=3))
        map_p = ctx.enter_context(tc.tile_pool(name="maps", bufs=2))
        kk_p = ctx.enter_context(tc.tile_pool(name="kk", bufs=2))
        sm_p = ctx.enter_context(tc.tile_pool(name="small", bufs=2))
        ps_kv = ctx.enter_context(tc.tile_pool(name="pskv", bufs=3, space="PSUM"))
        ps_q = ctx.enter_context(tc.tile_pool(name="psq", bufs=1, space="PSUM"))
        ps_acc = ctx.enter_context(tc.tile_pool(name="psacc", bufs=1, space="PSUM"))

        # ---- constants ----
        qw_t = const.tile([CIN, C], dcv, tag="qw")
        kw_t = const.tile([CIN, C], dcv, tag="kw")
        vw_t = const.tile([CIN, C], dcv, tag="vw")
        nc.sync.dma_start(qw_t[:], qw_d[:])
        nc.sync.dma_start(kw_t[:], kw_d[:])
        nc.sync.dma_start(vw_t[:], vw_d[:])
        ea_t = const.tile([128, W], f32, tag="ea")       # emb_a[c, w], both halves
        nc.sync.dma_start(ea_t[0:C, :], ea_d[:])
        nc.sync.dma_start(ea_t[C:128, :], ea_d[:])
        eb_t = const.tile([128, HH], f32, tag="eb")      # emb_b[c, 64*half + hl]
        nc.sync.dma_start(eb_t[0:C, :], _ap(eb_d, 0, [[1, HH]]))
        nc.sync.dma_start(eb_t[C:128, :], _ap(eb_d, HH, [[1, HH]]))
        ident = const.tile([128, 128], f32, tag="ident")
        masks.make_identity(nc, ident[:])
        idents = {f32: ident}
        for dt_ in {dkk, d_e2} - {f32}:
            it = const.tile([128, 128], dt_, tag=f"ident{dt_}")
            nc.vector.tensor_copy(it[:], ident[:])
            idents[dt_] = it

        for h0 in range(0, HH, ch):
            # ---- input chunk DMAs ----
            xp_t = xp_p.tile([CIN, 2 * RW], dcv, tag="xp")
            for half in (0, 1):
                nc.sync.dma_start(
                    xp_t[:, half * RW:(half + 1) * RW],
                    _ap(xp_d, (HH * half + h0) * WP, [[1, RW]]))
            mix_t = mix_p.tile([128, n], f32, tag="mix")
            for half in (0, 1):
                nc.sync.dma_start(
                    mix_t[C * half:C * (half + 1), :],
                    _ap(em_d, (HH * half + h0) * W, [[1, n]]))

            # ---- 1x1 convs on PE ----
            q_ps = ps_q.tile([128, n], f32, tag="q")
            for half in (0, 1):
                rhs = _ap(xp_t[:], half * RW + PAD * WP + PAD, [[WP, ch], [1, W]])
                nc.tensor.matmul(q_ps[C * half:C * (half + 1), :], qw_t[:], rhs,
                                 start=True, stop=True)
            kv_ps = {}
            for name, w_t in (("k", kw_t), ("v", vw_t)):
                for pc in range(0, RW, piece):
                    pw = min(piece, RW - pc)
                    pt = ps_kv.tile([128, piece], f32, tag="kv")
                    kv_ps[(name, pc)] = (pt, pw)
                    for half in (0, 1):
                        nc.tensor.matmul(
                            pt[C * half:C * (half + 1), 0:pw],
                            w_t[:], xp_t[:, half * RW + pc: half * RW + pc + pw],
                            start=True, stop=True)

            # ---- PSUM -> SBUF maps: k/v copies + v^2 on ACT ----
            # B variants hold the same map shifted right by one element so
            # odd-dj window reads stay 4-byte aligned (DVE 2x packing).
            k_map = map_p.tile([128, RW], dkk, tag="kmap")
            v_map = map_p.tile([128, RW], dkk, tag="vmap")
            v2_map = map_p.tile([128, RW], dkk, tag="v2map")
            for pc in range(0, RW, piece):
                pt, pw = kv_ps[("k", pc)]
                nc.scalar.copy(k_map[:, pc:pc + pw], pt[:, 0:pw])
                pt, pw = kv_ps[("v", pc)]
                nc.scalar.copy(v_map[:, pc:pc + pw], pt[:, 0:pw])
                nc.scalar.activation(v2_map[:, pc:pc + pw], pt[:, 0:pw], SQUARE)
            if split:
                k_b = map_p.tile([128, RW + 2], dkk, tag="kb")
                v_b = map_p.tile([128, RW + 2], dkk, tag="vb")
                v2_b = map_p.tile([128, RW + 2], dkk, tag="v2b")
                for a_t, b_t in ((k_map, k_b), (v_map, v_b), (v2_map, v2_b)):
                    nc.gpsimd.tensor_copy(b_t[:, 1:RW + 1], a_t[:, 0:RW])
            else:
                k_b = v_b = v2_b = None

            # ---- E = (emb_a + emb_b) * emb_mix ----
            tmp_t = sm_p.tile([128, n], f32, tag="tmpE")
            nc.vector.tensor_tensor(
                _ap(tmp_t[:], 0, [[W, ch], [1, W]]),
                _ap(ea_t[:], 0, [[0, ch], [1, W]]),
                _ap(eb_t[:], h0, [[1, ch], [0, W]]), ADD)
            E_t = sm_p.tile([128, n], dkk, tag="E")
            nc.vector.tensor_tensor(E_t[:], tmp_t[:], mix_t[:], MULT)

            # ---- KK-expanded stages ----
            # ISA: max 3 free dims -> one instr per di (fp32), or per
            # (di, dj-parity) when 16-bit so every operand stays 4B-aligned.
            def tt_kk(out_t, make_in0, make_in1):
                if not split:
                    for di in range(K):
                        nc.vector.tensor_tensor(
                            _ap(out_t[:], di * K * n, [[n, K], [W, ch], [1, W]]),
                            make_in0(di, None), make_in1(di, None), MULT)
                else:
                    for di in range(K):
                        for par in (0, 1):
                            nc.vector.tensor_tensor(
                                _ap(out_t[:], (di * K + par) * n,
                                    [[2 * n, 2], [W, ch], [1, W]]),
                                make_in0(di, par), make_in1(di, par), MULT)

            def win_di(m_a, m_b):
                def f(di, par):
                    if par is None:
                        return _ap(m_a[:], di * WP, [[1, K], [WP, ch], [1, W]])
                    src = m_a if par == 0 else m_b
                    return _ap(src[:], di * WP + 2 * par,
                               [[2, 2], [WP, ch], [1, W]])
                return f

            def bc_c(c_t):
                def f(di, par):
                    kdim = [0, K] if par is None else [0, 2]
                    return _ap(c_t[:], 0, [kdim, [W, ch], [1, W]])
                return f

            def kk_slice(k_t):
                def f(di, par):
                    if par is None:
                        return _ap(k_t[:], di * K * n, [[n, K], [W, ch], [1, W]])
                    return _ap(k_t[:], (di * K + par) * n,
                               [[2 * n, 2], [W, ch], [1, W]])
                return f

            def pe_reduce(src_t, acc_t, dt_):
                # acc[(half,c), pos] = sum_kk src[(half,c), kk*n + pos]
                # via 16 PSUM-accumulating identity matmuls (exact fp32 sums).
                for kk in range(KK):
                    nc.tensor.matmul(
                        acc_t[:], idents[dt_][:],
                        src_t[:, kk * n:(kk + 1) * n],
                        start=(kk == 0), stop=(kk == KK - 1))

            t1 = kk_p.tile([128, KK * n], dkk, tag="kkA")
            tt_kk(t1, win_di(v2_map, v2_b), bc_c(E_t))
            e1 = kk_p.tile([128, KK * n], dkk, tag="kkB")
            nc.scalar.activation(e1[:], t1[:], EXP)

            r1_ps = ps_acc.tile([128, n], f32, tag="r1")
            pe_reduce(e1, r1_ps, dkk)
            rc1 = sm_p.tile([128, n], f32, tag="rc1")
            nc.vector.reciprocal(rc1[:], r1_ps[:])
            qp_t = sm_p.tile([128, n], dkk, tag="qp")
            nc.vector.tensor_tensor(qp_t[:], q_ps[:], rc1[:], MULT)

            m1 = kk_p.tile([128, KK * n], dkk, tag="kkA")
            tt_kk(m1, win_di(k_map, k_b), kk_slice(e1))
            s2 = kk_p.tile([128, KK * n], dkk, tag="kkB")
            tt_kk(s2, kk_slice(m1), bc_c(qp_t))
            e2 = kk_p.tile([128, KK * n], d_e2, tag="kkC")
            nc.scalar.activation(e2[:], s2[:], EXP)

            r2_ps = ps_acc.tile([128, n], f32, tag="r2")
            pe_reduce(e2, r2_ps, d_e2)
            m2 = kk_p.tile([128, KK * n], dkk, tag="kkA")
            if cfg["e2_fp32"]:
                # fp32 e2 -> plain per-di instructions (1x mode anyway)
                for di in range(K):
                    nc.vector.tensor_tensor(
                        _ap(m2[:], di * K * n, [[n, K], [W, ch], [1, W]]),
                        _ap(e2[:], di * K * n, [[n, K], [W, ch], [1, W]]),
                        win_di(v_map, v_b)(di, None), MULT)
            else:
                tt_kk(m2, kk_slice(e2), win_di(v_map, v_b))
            r3_ps = ps_acc.tile([128, n], f32, tag="r3")
            pe_reduce(m2, r3_ps, dkk)

            rc2 = sm_p.tile([128, n], f32, tag="rc2")
            nc.vector.reciprocal(rc2[:], r2_ps[:])
            out_t = sm_p.tile([128, n], f32, tag="out")
            nc.vector.tensor_tensor(out_t[:], r3_ps[:], rc2[:], MULT)

            for half in (0, 1):
                nc.sync.dma_start(
                    _ap(out_d, (HH * half + h0) * W, [[1, n]]),
                    out_t[C * half:C * (half + 1), :])


_compiled_nc = None


def _get_nc():
    global _compiled_nc
    if _compiled_nc is None:
        nc = bacc.Bacc("TRN2", target_bir_lowering=False, debug=False,
                       num_devices=N_CORES)
        build_kernel(nc)
        nc.compile()
        _compiled_nc = nc
    return _compiled_nc


def _shard_inputs(x, q_w, k_w, v_w, emb_a, emb_b, emb_mix):
    cv_np = mybir.dt.np(CFG["conv"])
    xp = np.pad(x.astype(np.float32), ((0, 0), (0, 0), (PAD, PAD), (PAD, PAD)))
    xp = xp.astype(cv_np)
    common = {
        "q_wT": np.ascontiguousarray(q_w.T.astype(cv_np)),
        "k_wT": np.ascontiguousarray(k_w.T.astype(cv_np)),
        "v_wT": np.ascontiguousarray(v_w.T.astype(cv_np)),
        "emb_a": np.ascontiguousarray(emb_a.astype(np.float32)),
        "emb_b": np.ascontiguousarray(emb_b.astype(np.float32)),
        "emb_mix": np.ascontiguousarray(emb_mix.astype(np.float32).reshape(C, H * W)),
    }
    return [dict(common, xp=np.ascontiguousarray(xp[b].reshape(CIN, HP * WP)))
            for b in range(B)]


def kernel(x, q_w, k_w, v_w, emb_a, emb_b, emb_mix):
    nc = _get_nc()
    in_maps = _shard_inputs(x, q_w, k_w, v_w, emb_a, emb_b, emb_mix)
    res = bass_utils.run_bass_kernel_spmd(nc, in_maps, list(range(N_CORES)))
    out = np.stack([res.results[b]["out"].reshape(C, H, W) for b in range(B)])
    return out.astype(np.float32)


# revision 16
# speedup vs baseline: 2.6163x; 1.3574x over previous
"""Trainium2 Bass kernel for nn_AttentionStem (sparse local attention stem).

Math per output element (b, c, h, w), window kk = (di, dj) in 4x4, PAD=2:
  E[c,h,w]   = (emb_a[c,w] + emb_b[c,h]) * emb_mix[c,h,w]
  e1_kk      = exp(v_kk^2 * E)                  (softmax-1 numerator)
  q'         = q / sum_kk(e1)                   (fold softmax-1 denom into q)
  e2_kk      = exp(q' * k_kk * e1_kk)           (softmax-2 numerator)
  out        = sum_kk(e2 * v_kk) / sum_kk(e2)

Sharding: pure data parallel, one batch element per NeuronCore (8 cores).
Layout per core: SBUF partition p = 64*half + c  (half = h<64 ? 0 : 1),
free dims stream (h, w); KK tiles are [128, KK, n] with kk outermost.
The three sum_kk reductions run on the TensorEngine as chains of 16
accumulating 128x128 transposes into PSUM (exact fp32 sums); softmax-2's
divide happens in transposed space and one transpose brings the result back.
"""
import sys, os
for _p in ("/opt/trn_rl_repo", "/root/.axon_site/_ro/trn_rl_repo"):
    if os.path.isdir(_p) and _p not in sys.path:
        sys.path.insert(0, _p)

from contextlib import ExitStack
import numpy as np

import concourse.bass as bass
import concourse.bacc as bacc
import concourse.tile as tile
from concourse import mybir
import concourse.bass_utils as bass_utils
from concourse.bass_types import AP
from concourse import masks

N_CORES = 8
B, CIN, H, W = 8, 3, 128, 128
C = 64
K, PAD, KK = 4, 2, 16
HP, WP = H + 2 * PAD, W + 2 * PAD  # 132, 132
HH = H // 2                        # rows per half (64)

F32 = mybir.dt.float32
BF16 = mybir.dt.bfloat16
F32R = mybir.dt.float32r
MULT = mybir.AluOpType.mult
ADD = mybir.AluOpType.add
EXP = mybir.ActivationFunctionType.Exp
SQUARE = mybir.ActivationFunctionType.Square

CH = 2  # h-rows per half per chunk

# Precision / engine configuration.
#   kk:    dtype of the KK-expanded pipeline (maps, t1/e1/m1/s2/e2/m2)
#   conv:  dtype of the 1x1-conv matmuls (fp32: 4 cyc/row, fp32r/bf16: 1)
#   e2_fp32: keep softmax-2 numerators in fp32 (accuracy of the output path)
CFG = dict(kk=BF16, conv=F32R, e2_fp32=False)


def _ap(base: AP, offset: int, dims):
    """Build a custom free-dim AP on a tile/dram AP, keeping its partition dim."""
    return AP(tensor=base.tensor, offset=base.offset + offset,
              ap=[list(base.ap[0])] + [list(d) for d in dims])


def build_kernel(nc, ch: int = CH, cfg=None):
    cfg = dict(CFG if cfg is None else cfg)
    f32 = F32
    dkk = cfg["kk"]                 # dtype of KK pipeline
    d_e2 = f32 if cfg["e2_fp32"] else dkk
    dcv = cfg["conv"]               # conv matmul dtype
    split = dkk != f32              # parity-split DVE instrs for 2x mode
    n = ch * W                      # spatial elems per partition per chunk
    NG = n // 128                   # 128-wide position groups (= ch)
    RR = ch + K - 1                 # padded rows needed per half per chunk
    RW = RR * WP                    # map cols per half per chunk
    piece = (RW + 1) // 2 if RW > 512 else RW   # matmul col piece (<=512)

    xp_d = nc.dram_tensor("xp", [CIN, HP * WP], dcv, kind="ExternalInput").ap()
    qw_d = nc.dram_tensor("q_wT", [CIN, C], dcv, kind="ExternalInput").ap()
    kw_d = nc.dram_tensor("k_wT", [CIN, C], dcv, kind="ExternalInput").ap()
    vw_d = nc.dram_tensor("v_wT", [CIN, C], dcv, kind="ExternalInput").ap()
    ea_d = nc.dram_tensor("emb_a", [C, W], f32, kind="ExternalInput").ap()
    eb_d = nc.dram_tensor("emb_b", [C, H], f32, kind="ExternalInput").ap()
    em_d = nc.dram_tensor("emb_mix", [C, H * W], dkk, kind="ExternalInput").ap()
    out_d = nc.dram_tensor("out", [C, H * W], f32, kind="ExternalOutput").ap()

    with tile.TileContext(nc) as tc, ExitStack() as ctx:
        const = ctx.enter_context(tc.tile_pool(name="const", bufs=1))
        xp_p = ctx.enter_context(tc.tile_pool(name="xp", bufs=3))
        mix_p = ctx.enter_context(tc.tile_pool(name="mix", bufs=3))
        map_p = ctx.enter_context(tc.tile_pool(name="maps", bufs=2))
        kk_p = ctx.enter_context(tc.tile_pool(name="kk", bufs=2))
        sm_p = ctx.enter_context(tc.tile_pool(name="small", bufs=2))
        ps_kv = ctx.enter_context(tc.tile_pool(name="pskv", bufs=3, space="PSUM"))
        ps_q = ctx.enter_context(tc.tile_pool(name="psq", bufs=1, space="PSUM"))
        ps_acc = ctx.enter_context(tc.tile_pool(name="psacc", bufs=1, space="PSUM"))
        ps_acc2 = ctx.enter_context(tc.tile_pool(name="psacc2", bufs=2, space="PSUM"))

        # ---- constants ----
        qw_t = const.tile([CIN, C], dcv, tag="qw")
        kw_t = const.tile([CIN, C], dcv, tag="kw")
        vw_t = const.tile([CIN, C], dcv, tag="vw")
        nc.sync.dma_start(qw_t[:], qw_d[:])
        nc.sync.dma_start(kw_t[:], kw_d[:])
        nc.sync.dma_start(vw_t[:], vw_d[:])
        ea_t = const.tile([128, W], f32, tag="ea")       # emb_a[c, w], both halves
        nc.sync.dma_start(ea_t[0:C, :], ea_d[:])
        nc.sync.dma_start(ea_t[C:128, :], ea_d[:])
        eb_t = const.tile([128, HH], f32, tag="eb")      # emb_b[c, 64*half + hl]
        nc.sync.dma_start(eb_t[0:C, :], _ap(eb_d, 0, [[1, HH]]))
        nc.sync.dma_start(eb_t[C:128, :], _ap(eb_d, HH, [[1, HH]]))
        ident = const.tile([128, 128], f32, tag="ident")
        masks.make_identity(nc, ident[:])
        idents = {f32: ident}
        for dt_ in {dkk, d_e2} - {f32}:
            it = const.tile([128, 128], dt_, tag=f"ident{dt_}")
            nc.vector.tensor_copy(it[:], ident[:])
            idents[dt_] = it

        for h0 in range(0, HH, ch):
            # ---- input chunk DMAs ----
            xp_t = xp_p.tile([CIN, 2 * RW], dcv, tag="xp")
            for half in (0, 1):
                nc.sync.dma_start(
                    xp_t[:, half * RW:(half + 1) * RW],
                    _ap(xp_d, (HH * half + h0) * WP, [[1, RW]]))
            mix_t = mix_p.tile([128, n], dkk, tag="mix")
            for half in (0, 1):
                nc.sync.dma_start(
                    mix_t[C * half:C * (half + 1), :],
                    _ap(em_d, (HH * half + h0) * W, [[1, n]]))

            # ---- 1x1 convs on PE ----
            q_ps = ps_q.tile([128, n], f32, tag="q")
            for half in (0, 1):
                rhs = _ap(xp_t[:], half * RW + PAD * WP + PAD, [[WP, ch], [1, W]])
                nc.tensor.matmul(q_ps[C * half:C * (half + 1), :], qw_t[:], rhs,
                                 start=True, stop=True)
            kv_ps = {}
            for name, w_t in (("k", kw_t), ("v", vw_t)):
                for pc in range(0, RW, piece):
                    pw = min(piece, RW - pc)
                    pt = ps_kv.tile([128, piece], f32, tag="kv")
                    kv_ps[(name, pc)] = (pt, pw)
                    for half in (0, 1):
                        nc.tensor.matmul(
                            pt[C * half:C * (half + 1), 0:pw],
                            w_t[:], xp_t[:, half * RW + pc: half * RW + pc + pw],
                            start=True, stop=True)

            # ---- PSUM -> SBUF maps: k/v copies + v^2 on ACT ----
            # B variants hold the same map shifted right by one element so
            # odd-dj window reads stay 4-byte aligned (DVE 2x packing).
            k_map = map_p.tile([128, RW], dkk, tag="kmap")
            v_map = map_p.tile([128, RW], dkk, tag="vmap")
            v2_map = map_p.tile([128, RW], dkk, tag="v2map")
            for pc in range(0, RW, piece):
                pt, pw = kv_ps[("k", pc)]
                nc.scalar.copy(k_map[:, pc:pc + pw], pt[:, 0:pw])
                pt, pw = kv_ps[("v", pc)]
                nc.scalar.copy(v_map[:, pc:pc + pw], pt[:, 0:pw])
                nc.scalar.activation(v2_map[:, pc:pc + pw], pt[:, 0:pw], SQUARE)
            if split:
                k_b = map_p.tile([128, RW + 2], dkk, tag="kb")
                v_b = map_p.tile([128, RW + 2], dkk, tag="vb")
                v2_b = map_p.tile([128, RW + 2], dkk, tag="v2b")
                for a_t, b_t in ((k_map, k_b), (v_map, v_b), (v2_map, v2_b)):
                    nc.gpsimd.tensor_copy(b_t[:, 1:RW + 1], a_t[:, 0:RW])
            else:
                k_b = v_b = v2_b = None

            # ---- E = (emb_a + emb_b) * emb_mix ----
            tmp_t = sm_p.tile([128, n], f32, tag="tmpE")
            nc.gpsimd.tensor_tensor(
                _ap(tmp_t[:], 0, [[W, ch], [1, W]]),
                _ap(ea_t[:], 0, [[0, ch], [1, W]]),
                _ap(eb_t[:], h0, [[1, ch], [0, W]]), ADD)
            E_t = sm_p.tile([128, n], dkk, tag="E")
            nc.gpsimd.tensor_tensor(E_t[:], tmp_t[:], mix_t[:], MULT)

            # ---- KK-expanded stages ----
            # ISA: max 3 free dims -> one instr per di (fp32), or per
            # (di, dj-parity) when 16-bit so every operand stays 4B-aligned.
            def tt_kk(out_t, make_in0, make_in1):
                if not split:
                    for di in range(K):
                        nc.vector.tensor_tensor(
                            _ap(out_t[:], di * K * n, [[n, K], [W, ch], [1, W]]),
                            make_in0(di, None), make_in1(di, None), MULT)
                else:
                    for di in range(K):
                        for par in (0, 1):
                            nc.vector.tensor_tensor(
                                _ap(out_t[:], (di * K + par) * n,
                                    [[2 * n, 2], [W, ch], [1, W]]),
                                make_in0(di, par), make_in1(di, par), MULT)

            def win_di(m_a, m_b):
                def f(di, par):
                    if par is None:
                        return _ap(m_a[:], di * WP, [[1, K], [WP, ch], [1, W]])
                    src = m_a if par == 0 else m_b
                    return _ap(src[:], di * WP + 2 * par,
                               [[2, 2], [WP, ch], [1, W]])
                return f

            def bc_c(c_t):
                def f(di, par):
                    kdim = [0, K] if par is None else [0, 2]
                    return _ap(c_t[:], 0, [kdim, [W, ch], [1, W]])
                return f

            def kk_slice(k_t):
                def f(di, par):
                    if par is None:
                        return _ap(k_t[:], di * K * n, [[n, K], [W, ch], [1, W]])
                    return _ap(k_t[:], (di * K + par) * n,
                               [[2 * n, 2], [W, ch], [1, W]])
                return f

            def pe_reduce(src_t, acc_t, dt_):
                # acc[(half,c), pos] = sum_kk src[(half,c), kk*n + pos]
                # via 16 PSUM-accumulating identity matmuls (exact fp32 sums).
                for kk in range(KK):
                    nc.tensor.matmul(
                        acc_t[:], idents[dt_][:],
                        src_t[:, kk * n:(kk + 1) * n],
                        start=(kk == 0), stop=(kk == KK - 1))

            t1 = kk_p.tile([128, KK * n], dkk, tag="kkT1")
            tt_kk(t1, win_di(v2_map, v2_b), bc_c(E_t))
            e1 = kk_p.tile([128, KK * n], dkk, tag="kkE1")
            nc.scalar.activation(e1[:], t1[:], EXP)

            r1_ps = ps_acc.tile([128, n], f32, tag="r1")
            pe_reduce(e1, r1_ps, dkk)
            rc1 = sm_p.tile([128, n], f32, tag="rc1")
            nc.vector.reciprocal(rc1[:], r1_ps[:])
            qp_t = sm_p.tile([128, n], dkk, tag="qp")
            nc.vector.tensor_tensor(qp_t[:], q_ps[:], rc1[:], MULT)

            m1 = kk_p.tile([128, KK * n], dkk, tag="kkM1")
            tt_kk(m1, win_di(k_map, k_b), kk_slice(e1))
            s2 = kk_p.tile([128, KK * n], dkk, tag="kkS2")
            tt_kk(s2, kk_slice(m1), bc_c(qp_t))
            e2 = kk_p.tile([128, KK * n], d_e2, tag="kkE2")
            nc.scalar.activation(e2[:], s2[:], EXP)

            r2_ps = ps_acc.tile([128, n], f32, tag="r2")
            pe_reduce(e2, r2_ps, d_e2)
            m2 = kk_p.tile([128, KK * n], dkk, tag="kkM2")
            if cfg["e2_fp32"]:
                # fp32 e2 -> plain per-di instructions (1x mode anyway)
                for di in range(K):
                    nc.vector.tensor_tensor(
                        _ap(m2[:], di * K * n, [[n, K], [W, ch], [1, W]]),
                        _ap(e2[:], di * K * n, [[n, K], [W, ch], [1, W]]),
                        win_di(v_map, v_b)(di, None), MULT)
            else:
                tt_kk(m2, kk_slice(e2), win_di(v_map, v_b))
            r3_ps = ps_acc2.tile([128, n], f32, tag="r3")
            pe_reduce(m2, r3_ps, dkk)

            rc2 = sm_p.tile([128, n], f32, tag="rc2")
            nc.vector.reciprocal(rc2[:], r2_ps[:])
            out_t = sm_p.tile([128, n], f32, tag="out")
            nc.vector.tensor_tensor(out_t[:], r3_ps[:], rc2[:], MULT)

            for half in (0, 1):
                nc.sync.dma_start(
                    _ap(out_d, (HH * half + h0) * W, [[1, n]]),
                    out_t[C * half:C * (half + 1), :])


_compiled_nc = None


def _get_nc():
    global _compiled_nc
    if _compiled_nc is None:
        nc = bacc.Bacc("TRN2", target_bir_lowering=False, debug=False,
                       num_devices=N_CORES)
        build_kernel(nc)
        nc.compile()
        _compiled_nc = nc
    return _compiled_nc


def _shard_inputs(x, q_w, k_w, v_w, emb_a, emb_b, emb_mix):
    cv_np = mybir.dt.np(CFG["conv"])
    xp = np.pad(x.astype(np.float32), ((0, 0), (0, 0), (PAD, PAD), (PAD, PAD)))
    xp = xp.astype(cv_np)
    common = {
        "q_wT": np.ascontiguousarray(q_w.T.astype(cv_np)),
        "k_wT": np.ascontiguousarray(k_w.T.astype(cv_np)),
        "v_wT": np.ascontiguousarray(v_w.T.astype(cv_np)),
        "emb_a": np.ascontiguousarray(emb_a.astype(np.float32)),
        "emb_b": np.ascontiguousarray(emb_b.astype(np.float32)),
        "emb_mix": np.ascontiguousarray(emb_mix.reshape(C, H * W).astype(mybir.dt.np(CFG["kk"]))),
    }
    return [dict(common, xp=np.ascontiguousarray(xp[b].reshape(CIN, HP * WP)))
            for b in range(B)]


def kernel(x, q_w, k_w, v_w, emb_a, emb_b, emb_mix):
    nc = _get_nc()
    in_maps = _shard_inputs(x, q_w, k_w, v_w, emb_a, emb_b, emb_mix)
    res = bass_utils.run_bass_kernel_spmd(nc, in_maps, list(range(N_CORES)))
    out = np.stack([res.results[b]["out"].reshape(C, H, W) for b in range(B)])
    return out.astype(np.float32)


# revision 18
# speedup vs baseline: 3.2951x; 1.2594x over previous
"""Trainium2 Bass kernel for nn_AttentionStem (sparse local attention stem).

Math per output element (b, c, h, w), window kk = (di, dj) in 4x4, PAD=2:
  E[c,h,w]   = (emb_a[c,w] + emb_b[c,h]) * emb_mix[c,h,w]
  e1_kk      = exp(v_kk^2 * E)                  (softmax-1 numerator)
  q'         = q / sum_kk(e1)                   (fold softmax-1 denom into q)
  e2_kk      = exp(q' * k_kk * e1_kk)           (softmax-2 numerator)
  out        = sum_kk(e2 * v_kk) / sum_kk(e2)

Sharding: pure data parallel, one batch element per NeuronCore (8 cores).
Layout per core: SBUF partition p = 64*half + c  (half = h<64 ? 0 : 1),
free dims stream (h, w); KK tiles are [128, KK, n] with kk outermost.
The three sum_kk reductions run on the TensorEngine as chains of 16
accumulating 128x128 transposes into PSUM (exact fp32 sums); softmax-2's
divide happens in transposed space and one transpose brings the result back.
"""
import sys, os
for _p in ("/opt/trn_rl_repo", "/root/.axon_site/_ro/trn_rl_repo"):
    if os.path.isdir(_p) and _p not in sys.path:
        sys.path.insert(0, _p)

from contextlib import ExitStack
import numpy as np

import concourse.bass as bass
import concourse.bacc as bacc
import concourse.tile as tile
from concourse import mybir
import concourse.bass_utils as bass_utils
from concourse.bass_types import AP
from concourse import masks

N_CORES = 8
B, CIN, H, W = 8, 3, 128, 128
C = 64
K, PAD, KK = 4, 2, 16
HP, WP = H + 2 * PAD, W + 2 * PAD  # 132, 132
HH = H // 2                        # rows per half (64)

F32 = mybir.dt.float32
BF16 = mybir.dt.bfloat16
F32R = mybir.dt.float32r
MULT = mybir.AluOpType.mult
ADD = mybir.AluOpType.add
EXP = mybir.ActivationFunctionType.Exp
SQUARE = mybir.ActivationFunctionType.Square

CH = 2  # h-rows per half per chunk

# Precision / engine configuration.
#   kk:    dtype of the KK-expanded pipeline (maps, t1/e1/m1/s2/e2/m2)
#   conv:  dtype of the 1x1-conv matmuls (fp32: 4 cyc/row, fp32r/bf16: 1)
#   e2_fp32: keep softmax-2 numerators in fp32 (accuracy of the output path)
CFG = dict(kk=BF16, conv=F32R, e2_fp32=False, mh=8,
           pool_tt=(('t1', 3), ('m2', 3)))


def _ap(base: AP, offset: int, dims):
    """Build a custom free-dim AP on a tile/dram AP, keeping its partition dim."""
    return AP(tensor=base.tensor, offset=base.offset + offset,
              ap=[list(base.ap[0])] + [list(d) for d in dims])


def build_kernel(nc, ch: int = CH, cfg=None):
    cfg = dict(CFG if cfg is None else cfg)
    f32 = F32
    dkk = cfg["kk"]                 # dtype of KK pipeline
    d_e2 = f32 if cfg["e2_fp32"] else dkk
    dcv = cfg["conv"]               # conv matmul dtype
    split = dkk != f32              # parity-split DVE instrs for 2x mode
    pool_tt = set(cfg.get("pool_tt") or ())
    n = ch * W                      # spatial elems per partition per chunk
    mh = cfg.get("mh", 8)           # map super-chunk rows per half
    RWm = (mh + K - 1) * WP         # map cols per half per super-chunk
    piece = -(-RWm // -(-RWm // 512))           # matmul col piece (<=512)

    xp_d = nc.dram_tensor("xp", [CIN, HP * WP], dcv, kind="ExternalInput").ap()
    qw_d = nc.dram_tensor("q_wT", [CIN, C], dcv, kind="ExternalInput").ap()
    kw_d = nc.dram_tensor("k_wT", [CIN, C], dcv, kind="ExternalInput").ap()
    vw_d = nc.dram_tensor("v_wT", [CIN, C], dcv, kind="ExternalInput").ap()
    ea_d = nc.dram_tensor("emb_a", [C, W], f32, kind="ExternalInput").ap()
    eb_d = nc.dram_tensor("emb_b", [C, H], f32, kind="ExternalInput").ap()
    em_d = nc.dram_tensor("emb_mix", [C, H * W], dkk, kind="ExternalInput").ap()
    out_d = nc.dram_tensor("out", [C, H * W], f32, kind="ExternalOutput").ap()

    with tile.TileContext(nc) as tc, ExitStack() as ctx:
        const = ctx.enter_context(tc.tile_pool(name="const", bufs=1))
        xp_p = ctx.enter_context(tc.tile_pool(name="xp", bufs=3))
        mix_p = ctx.enter_context(tc.tile_pool(name="mix", bufs=3))
        map_p = ctx.enter_context(tc.tile_pool(name="maps", bufs=2))
        kk_p = ctx.enter_context(tc.tile_pool(name="kk", bufs=2))
        sm_p = ctx.enter_context(tc.tile_pool(name="small", bufs=2))
        ps_kv = ctx.enter_context(tc.tile_pool(name="pskv", bufs=3, space="PSUM"))
        ps_q = ctx.enter_context(tc.tile_pool(name="psq", bufs=1, space="PSUM"))
        ps_acc = ctx.enter_context(tc.tile_pool(name="psacc", bufs=1, space="PSUM"))
        ps_acc2 = ctx.enter_context(tc.tile_pool(name="psacc2", bufs=2, space="PSUM"))

        # ---- constants ----
        qw_t = const.tile([CIN, C], dcv, tag="qw")
        kw_t = const.tile([CIN, C], dcv, tag="kw")
        vw_t = const.tile([CIN, C], dcv, tag="vw")
        nc.sync.dma_start(qw_t[:], qw_d[:])
        nc.sync.dma_start(kw_t[:], kw_d[:])
        nc.sync.dma_start(vw_t[:], vw_d[:])
        ea_t = const.tile([128, W], f32, tag="ea")       # emb_a[c, w], both halves
        nc.sync.dma_start(ea_t[0:C, :], ea_d[:])
        nc.sync.dma_start(ea_t[C:128, :], ea_d[:])
        eb_t = const.tile([128, HH], f32, tag="eb")      # emb_b[c, 64*half + hl]
        nc.sync.dma_start(eb_t[0:C, :], _ap(eb_d, 0, [[1, HH]]))
        nc.sync.dma_start(eb_t[C:128, :], _ap(eb_d, HH, [[1, HH]]))
        ident = const.tile([128, 128], f32, tag="ident")
        masks.make_identity(nc, ident[:])
        idents = {f32: ident}
        for dt_ in {dkk, d_e2} - {f32}:
            it = const.tile([128, 128], dt_, tag=f"ident{dt_}")
            nc.vector.tensor_copy(it[:], ident[:])
            idents[dt_] = it

        for mh0 in range(0, HH, mh):
            # ==== super-chunk: produce k/v/v^2 maps for mh rows per half ====
            xp_t = xp_p.tile([CIN, 2 * RWm], dcv, tag="xp")
            for half in (0, 1):
                nc.sync.dma_start(
                    xp_t[:, half * RWm:(half + 1) * RWm],
                    _ap(xp_d, (HH * half + mh0) * WP, [[1, RWm]]))

            kv_ps = {}
            for name, w_t in (("k", kw_t), ("v", vw_t)):
                for pc in range(0, RWm, piece):
                    pw = min(piece, RWm - pc)
                    pt = ps_kv.tile([128, 512], f32, tag="kv")
                    kv_ps[(name, pc)] = (pt, pw)
                    for half in (0, 1):
                        nc.tensor.matmul(
                            pt[C * half:C * (half + 1), 0:pw],
                            w_t[:], xp_t[:, half * RWm + pc: half * RWm + pc + pw],
                            start=True, stop=True)

            # PSUM -> SBUF maps (ACT); B variants shifted one element right so
            # odd-dj window reads stay 4-byte aligned for the DVE 2x mode.
            k_map = map_p.tile([128, RWm], dkk, tag="kmap")
            v_map = map_p.tile([128, RWm], dkk, tag="vmap")
            v2_map = map_p.tile([128, RWm], dkk, tag="v2map")
            for pc in range(0, RWm, piece):
                pt, pw = kv_ps[("k", pc)]
                nc.scalar.copy(k_map[:, pc:pc + pw], pt[:, 0:pw])
                pt, pw = kv_ps[("v", pc)]
                nc.scalar.copy(v_map[:, pc:pc + pw], pt[:, 0:pw])
                nc.scalar.activation(v2_map[:, pc:pc + pw], pt[:, 0:pw], SQUARE)
            if split:
                k_b = map_p.tile([128, RWm + 2], dkk, tag="kb")
                v_b = map_p.tile([128, RWm + 2], dkk, tag="vb")
                v2_b = map_p.tile([128, RWm + 2], dkk, tag="v2b")
                for a_t, b_t in ((k_map, k_b), (v_map, v_b), (v2_map, v2_b)):
                    nc.gpsimd.tensor_copy(b_t[:, 1:RWm + 1], a_t[:, 0:RWm])
            else:
                k_b = v_b = v2_b = None

            for h0 in range(mh0, mh0 + mh, ch):
                ro = (h0 - mh0) * WP       # row offset into the map tiles
                mix_t = mix_p.tile([128, n], dkk, tag="mix")
                for half in (0, 1):
                    nc.sync.dma_start(
                        mix_t[C * half:C * (half + 1), :],
                        _ap(em_d, (HH * half + h0) * W, [[1, n]]))

                q_ps = ps_q.tile([128, 512], f32, tag="q")
                for half in (0, 1):
                    rhs = _ap(xp_t[:],
                              half * RWm + (h0 - mh0 + PAD) * WP + PAD,
                              [[WP, ch], [1, W]])
                    nc.tensor.matmul(q_ps[C * half:C * (half + 1), 0:n], qw_t[:],
                                     rhs, start=True, stop=True)

                # ---- E = (emb_a + emb_b) * emb_mix  (on GPSIMD) ----
                tmp_t = sm_p.tile([128, n], f32, tag="tmpE")
                nc.gpsimd.tensor_tensor(
                    _ap(tmp_t[:], 0, [[W, ch], [1, W]]),
                    _ap(ea_t[:], 0, [[0, ch], [1, W]]),
                    _ap(eb_t[:], h0, [[1, ch], [0, W]]), ADD)
                E_t = sm_p.tile([128, n], dkk, tag="E")
                nc.gpsimd.tensor_tensor(E_t[:], tmp_t[:], mix_t[:], MULT)

                # ---- KK-expanded stages ----
                # ISA: max 3 free dims -> one instr per di (fp32), or per
                # (di, dj-parity) when 16-bit (keeps every operand 4B-aligned).
                def tt_kk(op_name, out_t, make_in0, make_in1):
                    if not split:
                        for di in range(K):
                            nc.vector.tensor_tensor(
                                _ap(out_t[:], di * K * n,
                                    [[n, K], [W, ch], [1, W]]),
                                make_in0(di, None), make_in1(di, None), MULT)
                    else:
                        for di in range(K):
                            eng = (nc.gpsimd if (op_name, di) in pool_tt
                                   else nc.vector)
                            for par in (0, 1):
                                eng.tensor_tensor(
                                    _ap(out_t[:], (di * K + par) * n,
                                        [[2 * n, 2], [W, ch], [1, W]]),
                                    make_in0(di, par), make_in1(di, par), MULT)

                def win_di(m_a, m_b):
                    def f(di, par):
                        if par is None:
                            return _ap(m_a[:], ro + di * WP,
                                       [[1, K], [WP, ch], [1, W]])
                        src = m_a if par == 0 else m_b
                        return _ap(src[:], ro + di * WP + 2 * par,
                                   [[2, 2], [WP, ch], [1, W]])
                    return f

                def bc_c(c_t):
                    def f(di, par):
                        kdim = [0, K] if par is None else [0, 2]
                        return _ap(c_t[:], 0, [kdim, [W, ch], [1, W]])
                    return f

                def kk_slice(k_t):
                    def f(di, par):
                        if par is None:
                            return _ap(k_t[:], di * K * n,
                                       [[n, K], [W, ch], [1, W]])
                        return _ap(k_t[:], (di * K + par) * n,
                                   [[2 * n, 2], [W, ch], [1, W]])
                    return f

                def pe_reduce(src_t, acc_t, dt_):
                    # acc[(half,c), pos] = sum_kk src[(half,c), kk*n + pos]
                    # via 16 PSUM-accumulating identity matmuls (exact fp32).
                    for kk in range(KK):
                        nc.tensor.matmul(
                            acc_t[:, 0:n], idents[dt_][:],
                            src_t[:, kk * n:(kk + 1) * n],
                            start=(kk == 0), stop=(kk == KK - 1))

                t1 = kk_p.tile([128, KK * n], dkk, tag="kkT1")
                tt_kk("t1", t1, win_di(v2_map, v2_b), bc_c(E_t))
                e1 = kk_p.tile([128, KK * n], dkk, tag="kkE1")
                nc.scalar.activation(e1[:], t1[:], EXP)

                r1_ps = ps_acc.tile([128, 512], f32, tag="r1")
                pe_reduce(e1, r1_ps, dkk)
                rc1 = sm_p.tile([128, n], f32, tag="rc1")
                nc.vector.reciprocal(rc1[:], r1_ps[:, 0:n])
                qp_t = sm_p.tile([128, n], dkk, tag="qp")
                nc.vector.tensor_tensor(qp_t[:], q_ps[:, 0:n], rc1[:], MULT)

                m1 = kk_p.tile([128, KK * n], dkk, tag="kkM1")
                tt_kk("m1", m1, win_di(k_map, k_b), kk_slice(e1))
                s2 = kk_p.tile([128, KK * n], dkk, tag="kkS2")
                tt_kk("s2", s2, kk_slice(m1), bc_c(qp_t))
                e2 = kk_p.tile([128, KK * n], d_e2, tag="kkE2")
                nc.scalar.activation(e2[:], s2[:], EXP)

                r2_ps = ps_acc.tile([128, 512], f32, tag="r2")
                pe_reduce(e2, r2_ps, d_e2)
                m2 = kk_p.tile([128, KK * n], dkk, tag="kkM2")
                if cfg["e2_fp32"]:
                    for di in range(K):
                        nc.vector.tensor_tensor(
                            _ap(m2[:], di * K * n, [[n, K], [W, ch], [1, W]]),
                            _ap(e2[:], di * K * n, [[n, K], [W, ch], [1, W]]),
                            win_di(v_map, v_b)(di, None), MULT)
                else:
                    tt_kk("m2", m2, kk_slice(e2), win_di(v_map, v_b))
                r3_ps = ps_acc2.tile([128, 512], f32, tag="r3")
                pe_reduce(m2, r3_ps, dkk)

                rc2 = sm_p.tile([128, n], f32, tag="rc2")
                nc.vector.reciprocal(rc2[:], r2_ps[:, 0:n])
                out_t = sm_p.tile([128, n], f32, tag="out")
                nc.vector.tensor_tensor(out_t[:], r3_ps[:, 0:n], rc2[:], MULT)

                for half in (0, 1):
                    nc.sync.dma_start(
                        _ap(out_d, (HH * half + h0) * W, [[1, n]]),
                        out_t[C * half:C * (half + 1), :])


_compiled_nc = None


def _get_nc():
    global _compiled_nc
    if _compiled_nc is None:
        nc = bacc.Bacc("TRN2", target_bir_lowering=False, debug=False,
                       num_devices=N_CORES)
        build_kernel(nc)
        nc.compile()
        _compiled_nc = nc
    return _compiled_nc


def _shard_inputs(x, q_w, k_w, v_w, emb_a, emb_b, emb_mix):
    cv_np = mybir.dt.np(CFG["conv"])
    xp = np.pad(x.astype(np.float32), ((0, 0), (0, 0), (PAD, PAD), (PAD, PAD)))
    xp = xp.astype(cv_np)
    common = {
        "q_wT": np.ascontiguousarray(q_w.T.astype(cv_np)),
        "k_wT": np.ascontiguousarray(k_w.T.astype(cv_np)),
        "v_wT": np.ascontiguousarray(v_w.T.astype(cv_np)),
        "emb_a": np.ascontiguousarray(emb_a.astype(np.float32)),
        "emb_b": np.ascontiguousarray(emb_b.astype(np.float32)),
        "emb_mix": np.ascontiguousarray(emb_mix.reshape(C, H * W).astype(mybir.dt.np(CFG["kk"]))),
    }
    return [dict(common, xp=np.ascontiguousarray(xp[b].reshape(CIN, HP * WP)))
            for b in range(B)]


def kernel(x, q_w, k_w, v_w, emb_a, emb_b, emb_mix):
    nc = _get_nc()
    in_maps = _shard_inputs(x, q_w, k_w, v_w, emb_a, emb_b, emb_mix)
    res = bass_utils.run_bass_kernel_spmd(nc, in_maps, list(range(N_CORES)))
    out = np.stack([res.results[b]["out"].reshape(C, H, W) for b in range(B)])
    return out.astype(np.float32)
